# revision 15
# baseline (speedup 1.0000x reference)
"""Trainium2 Bass kernel for nn_MESNReadout (multi-layer echo state network readout).

Strategy
--------
Pure data parallelism over batch: B=512 -> 64 rows per core on 8 cores; all
weights replicated; output gathered on host.

The reference is a T=1024 sequential scan with L=3 stacked reservoir layers
plus a leaky-integrator side state xv. We reformulate with a *layer-skewed
wavefront*: wavefront k computes x0(k), x1(k-1), x2(k-2), hv(k-3)
simultaneously, where hv(t) = tanh(zv(t)) is the inner tanh of the xv
update. Every input a wavefront needs then comes from the previous
wavefront's tanh output T_{k-1} plus a staged history [x0(k-4); x1(k-4);
x2(k-4)] for the xv pooling term (DVE copies aggregate three rb slots so
PE sees ONE pool matmul; three tiny matmuls instead serialize on the PE
h64 column group at ~200ns each and blow the period).

One wavefront is:
  PE:  projA/projB (input projections, PSUM slot init, prefetched PF ahead)
       mm_b  (pool history -> zv rows, off critical path)
       mm_a  (recurrent matmul, the only op on the dependent chain)
  ACT: one tanh PSUM->SBUF (bf16 out)
  DVE: three small history copies (a wavefront of slack)

The critical cycle is mm_a -> tanh -> mm_a: the minimal PE->ACT->PE round
trip this recurrence permits (~640ns steady: 52 sem + 215 matmul + 58 sem
+ 314 tanh, all but ~95ns of which is fixed engine pipeline latency).
Everything is bf16 so fp32's 2-pass matmuls are avoided and the bigwa
weight reload stays off the chain (the tanh wait rides on the MATMUL,
letting LDWEIGHTS dual-issue under the preceding matmuls). Splitting the
batch columns to pipeline tanh/matmul halves does NOT work: a PSUM bank
read while the bank's accumulation group is open serializes against the
other half's matmul, doubling the round trips per wavefront.

State layout is transposed ([feature, batch]) so matmuls contract over
partitions, and *padded* to partition-aligned blocks x0@[0:20] x1@[32:52]
x2@[64:84] hv@[96:108] because engines can only address SBUF partition
ranges starting at 0/32/64/96 and matmul outputs must start at PSUM
partition 0/32/64/96. Gap rows carry zeros (weights are zero-padded). The
host pre-packs u into a paired time-shifted array up[128, T+5, 64] (rows
0:64 = uT(j-2), rows 64:128 = uT(j-3)) so one projection matmul covers two
skewed time blocks and boundary conditions fall out as zeros.
"""
import sys

import numpy as np

sys.path.insert(0, "/opt/trn_rl_repo")

L, S, TH, D = 3, 4, 5, 64
NCLS = 100
B = 512
DELTA = 0.9
NCORES = 8
BC = B // NCORES            # 64 batch rows per core
R = L * S * TH              # 60
LS = L * S                  # 12
F = R + LS                  # 72 logical state rows
SS = 108                    # padded state span
NB = 6                      # rotating state buffers
NS = 8                      # rotating PSUM slots: one full 2KB bank each
PF = 4                      # projection prefetch distance (slots ahead)
UCHUNK = 32                 # timesteps of `up` per DMA chunk

# padded positions of the 72 logical rows [x0(20) x1(20) x2(20) hv(12)]
NEWPOS = np.concatenate([np.arange(0, 20), np.arange(32, 52),
                         np.arange(64, 84), np.arange(96, 108)])


def _bd(Ws):
    a, b = Ws.shape[1], Ws.shape[2]
    M = np.zeros((S * a, S * b), np.float32)
    for s in range(S):
        M[s * a:(s + 1) * a, s * b:(s + 1) * b] = Ws[s]
    return M


def _hstack_s(Ws):
    return np.concatenate([Ws[s] for s in range(S)], axis=1).astype(np.float32)


def build_host_mats(W_in0, W_in_rest, W, Wv_in, Wv, W_out):
    MpT = np.zeros((LS, R), np.float32)
    for d in range(L):
        for s in range(S):
            MpT[4 * d + s, 20 * d + 5 * s:20 * d + 5 * s + TH] = 1.0 / TH

    # compact [72,72] recurrent matrix in logical order [x0 x1 x2 hv]
    Wc = np.zeros((F, F), np.float32)
    Wc[0:20, 0:20] = _bd(W[0])
    Wc[0:20, 20:40] = _bd(W_in_rest[0][:, D:, :])
    Wc[20:40, 20:40] = _bd(W[1])
    Wc[20:40, 40:60] = _bd(W_in_rest[1][:, D:, :])
    Wc[40:60, 40:60] = _bd(W[2])
    Wc[60:72, 60:72] = DELTA * Wv.T
    BigWa = np.zeros((SS, SS), np.float32)
    BigWa[np.ix_(NEWPOS, NEWPOS)] = Wc

    # input projections: WA -> out rows [0:52] = [U0 | gap | U1],
    # WB -> out rows [64:108] = [U2 | gap | Uv]
    WA = np.zeros((128, 52), np.float32)
    WA[0:64, 0:20] = _hstack_s(W_in0)
    WA[64:128, 32:52] = _hstack_s(W_in_rest[0][:, :D, :])
    WB = np.zeros((128, 44), np.float32)
    WB[0:64, 0:20] = _hstack_s(W_in_rest[1][:, :D, :])
    WB[64:128, 32:44] = Wv_in.T.astype(np.float32)

    # pool-history -> zv: out rows [64:108], cols 32:44 live
    Gw = ((1.0 - DELTA) * (Wv @ MpT)).T.astype(np.float32)   # [60, 12]
    gpool = np.zeros((96, 44), np.float32)
    gpool[0:20, 32:44] = Gw[0:20]
    gpool[32:52, 32:44] = Gw[20:40]
    gpool[64:84, 32:44] = Gw[40:60]

    # xv(T-1) = 0.1*pool(x(T-1)) + 0.9*hv(T-1) over padded feats rows
    poolhv = np.zeros((SS, LS), np.float32)
    poolhv[NEWPOS[0:60], :] = (1.0 - DELTA) * MpT.T
    poolhv[96:108, :] = DELTA * np.eye(LS, dtype=np.float32)

    woutp = np.zeros((SS, NCLS), np.float32)
    woutp[NEWPOS, :] = W_out.astype(np.float32)

    # pack all weights (incl. the bf16 readout mats) into one tensor so
    # startup is 2 DMA issues, not 7
    wpack = np.zeros((128, 360), np.float32)
    wpack[0:SS, 0:108] = BigWa
    wpack[0:96, 108:152] = gpool
    wpack[:, 152:204] = WA
    wpack[:, 204:248] = WB
    wpack[0:SS, 248:260] = poolhv
    wpack[0:SS, 260:360] = woutp
    fpack = np.zeros((128, 1), np.float32)  # b_out, filled by caller
    return wpack, fpack


def build_up(u_core, T):
    """u_core [BC, T, 64] -> up [128, T+5, BC] f32 (paired, shifted, padded)."""
    uT = np.ascontiguousarray(u_core.transpose(2, 1, 0)).astype(np.float32)
    up = np.zeros((128, T + 5, u_core.shape[0]), np.float32)
    up[0:64, 2:T + 2] = uT
    up[64:128, 3:T + 3] = uT
    return np.ascontiguousarray(up)


def build_nc(T, prec="bf16all", split=1):
    import concourse.bacc as bacc
    import concourse.mybir as mybir
    from concourse.tile import TileContext

    dt = mybir.dt.float32
    dtb = mybir.dt.bfloat16 if prec in ("bf16", "bf16all") else mybir.dt.float32
    dtu = mybir.dt.bfloat16 if prec == "bf16all" else mybir.dt.float32
    NW = T + 3
    NUP = T + 5

    nc = bacc.Bacc(None)
    up_d = nc.dram_tensor("up", [128, NUP, BC], dtu, kind="ExternalInput")
    wpack_d = nc.dram_tensor("wpack", [128, 360], dtb, kind="ExternalInput")
    fpack_d = nc.dram_tensor("fpack", [128, 1], dt, kind="ExternalInput")
    out_d = nc.dram_tensor("out", [NCLS, BC], dt, kind="ExternalOutput")

    with TileContext(nc) as tc:
        with (
            tc.tile_pool(name="const", bufs=1) as cpool,
            tc.tile_pool(name="ubuf", bufs=6) as upool,
            tc.tile_pool(name="state", bufs=1) as spool,
            tc.tile_pool(name="psum", bufs=1, space="PSUM") as ppool,
        ):
            wpack = cpool.tile([128, 360], dtb)
            fpack = cpool.tile([128, 1], dt)
            nc.sync.dma_start(wpack[:], wpack_d[:])
            # fpack is tail-only; issue it on ACT (idle at startup) so the
            # first u chunks aren't queued behind it on sync/gpsimd
            nc.scalar.dma_start(fpack[:], fpack_d[:])
            bigwa = wpack[0:SS, 0:108]
            gpool = wpack[0:96, 108:152]
            wa = wpack[:, 152:204]
            wb = wpack[:, 204:248]
            poolhv = wpack[0:SS, 248:260]
            wout = wpack[0:SS, 260:360]
            bout = fpack[0:NCLS, 0:1]

            # one PSUM region: slot j = one full 2KB bank, cols 0:BC used.
            # Only the alignment-gap rows (never written by any matmul but
            # read by the tanh) need zeroing; matmul start=True covers the
            # rest. These memsets gate the first projection (PSUM write
            # ordering), so issue them first on Vector.
            psum = ppool.tile([128, NS, 512], dt)
            nc.vector.memset(psum[32:64, :, 0:BC], 0.0)
            nc.vector.memset(psum[64:96, :, 0:BC], 0.0)

            # rb[:, j%NB, :] = T_{j-1} (tanh output of wavefront j-1), padded
            rb = spool.tile([SS, NB, BC], dtb)
            # hist[:, j%NB, :] = [x0(j-4) | gap | x1(j-4) | gap | x2(j-4)]
            hist = spool.tile([96, NB, BC], dtb)
            nc.vector.memset(rb[:], 0.0)
            nc.vector.memset(hist[:], 0.0)

            # variable-size chunks: small at the head so wavefront 0 isn't
            # gated on a large DMA
            chunks = []
            j = 0
            for w in (2, 2, 4, 8):
                if j < NUP:
                    chunks.append((j, min(w, NUP - j)))
                    j += w
            while j < NUP:
                w = min(UCHUNK, NUP - j)
                chunks.append((j, w))
                j += w
            j2c = {}
            for ci, (j0, w) in enumerate(chunks):
                for jj in range(j0, j0 + w):
                    j2c[jj] = ci
            u_tiles = [None] * len(chunks)
            # keep DMA issuance off ACT (on the critical chain) and PE.
            # The first two chunks gate wavefront 0: put them on gpsimd so
            # they race the wpack DMA on sync instead of queuing behind it.
            dma_eng = [nc.sync, nc.gpsimd]
            next_load = [0]

            def ensure_loaded(jmax):
                while (next_load[0] < len(chunks)
                       and chunks[next_load[0]][0] <= jmax):
                    ci = next_load[0]
                    j0, w = chunks[ci]
                    t = upool.tile([128, UCHUNK, BC], dtu, tag="uc")
                    eng = nc.gpsimd if ci < 2 else dma_eng[ci % 2]
                    eng.dma_start(t[:, :w, :], up_d[:, j0:j0 + w, :])
                    u_tiles[ci] = t
                    next_load[0] += 1

            def up_ap(j):
                ci = j2c[j]
                return u_tiles[ci][:, j - chunks[ci][0], :]

            def emit_proj(k):
                if k >= NW:
                    return
                sl = psum[:, k % NS, 0:BC]
                nc.tensor.matmul(sl[0:52, :], wa, up_ap(k + 2),
                                 start=True, stop=False, skip_group_check=True)
                nc.tensor.matmul(sl[64:108, :], wb, up_ap(k),
                                 start=True, stop=False, skip_group_check=True)

            ensure_loaded(PF + 2 + 2 * UCHUNK)
            for k in range(PF):
                emit_proj(k)

            HB = BC // split
            for k in range(NW):
                ensure_loaded(k + PF + 2 + 2 * UCHUNK)
                emit_proj(k + PF)
                sl = psum[:, k % NS, 0:BC]
                # xv pooling term from staged history (off critical path).
                # One matmul: three tiny matmuls from older rb slots instead
                # serialize on the PE h64 column group and blow the period.
                nc.tensor.matmul(sl[64:108, :], gpool, hist[:, k % NB, :],
                                 start=False, stop=False, skip_group_check=True)
                # the recurrent matmul + tanh, in `split` batch-column
                # halves so the tanh of one half overlaps the matmul of
                # the next (the dependent chain is per batch column)
                for h in range(split):
                    cs = slice(h * HB, (h + 1) * HB)
                    nc.tensor.matmul(sl[0:SS, cs], bigwa,
                                     rb[:, k % NB, cs],
                                     start=False, stop=(h == split - 1),
                                     skip_group_check=True)
                    nc.scalar.activation(rb[:, (k + 1) % NB, cs],
                                         sl[0:SS, cs],
                                         mybir.ActivationFunctionType.Tanh)
                # stage history: x0/x1 two slots ahead (extra slack),
                # x2 one ahead (its source is only ready then)
                if k + 2 < NW:
                    nc.vector.tensor_copy(hist[0:20, (k + 2) % NB, :],
                                          rb[0:20, (k - 1) % NB, :])
                    nc.vector.tensor_copy(hist[32:52, (k + 2) % NB, :],
                                          rb[32:52, k % NB, :])
                if k + 1 < NW:
                    nc.vector.tensor_copy(hist[64:84, (k + 1) % NB, :],
                                          rb[64:84, k % NB, :])

            # ---- tail: feats = [x0|x1|x2|xv](T-1) padded, then readout ----
            feats = spool.tile([SS, BC], dtb)
            nc.vector.memset(feats[:], 0.0)
            nc.vector.tensor_copy(feats[0:20, :], rb[0:20, T % NB, :])
            nc.vector.tensor_copy(feats[32:52, :], rb[32:52, (T + 1) % NB, :])
            nc.vector.tensor_copy(feats[64:84, :], rb[64:84, (T + 2) % NB, :])
            nc.vector.tensor_copy(feats[96:108, :], rb[96:108, (T + 3) % NB, :])
            nc.tensor.matmul(psum[0:LS, 0, 0:BC], poolhv, feats[0:SS, :],
                             start=True, stop=True, skip_group_check=True)
            nc.vector.tensor_copy(feats[96:108, :], psum[0:LS, 0, 0:BC])
            nc.tensor.matmul(psum[0:NCLS, 1, 0:BC], wout, feats[0:SS, :],
                             start=True, stop=True, skip_group_check=True)
            out_sb = spool.tile([NCLS, BC], dt)
            nc.scalar.activation(out_sb[:], psum[0:NCLS, 1, 0:BC],
                                 mybir.ActivationFunctionType.Identity,
                                 bias=bout)
            nc.sync.dma_start(out_d[:], out_sb[:])

    nc.compile()
    return nc


_NC_CACHE = {}


def _get_nc(T, prec="bf16all", split=1):
    key = (T, prec, split)
    if key not in _NC_CACHE:
        _NC_CACHE[key] = build_nc(T, prec, split)
    return _NC_CACHE[key]


def kernel(u, W_in0, W_in_rest, W, Wv_in, Wv, W_out, b_out,
           _T=None, _trace=False, _prec="bf16all", _split=1):
    from concourse.bass_utils import run_bass_kernel_spmd
    import ml_dtypes

    u = np.asarray(u, np.float32)
    T = _T or u.shape[1]
    cb = (lambda x: np.ascontiguousarray(x.astype(ml_dtypes.bfloat16))) \
        if _prec in ("bf16", "bf16all") else (lambda x: x)
    cu = (lambda x: np.ascontiguousarray(x.astype(ml_dtypes.bfloat16))) \
        if _prec == "bf16all" else (lambda x: x)
    wpack, fpack = build_host_mats(
        np.asarray(W_in0, np.float32), np.asarray(W_in_rest, np.float32),
        np.asarray(W, np.float32), np.asarray(Wv_in, np.float32),
        np.asarray(Wv, np.float32), np.asarray(W_out, np.float32))
    fpack = fpack.copy()
    fpack[0:NCLS, 0] = np.asarray(b_out, np.float32)

    nc = _get_nc(T, _prec, _split)
    in_maps = []
    for c in range(NCORES):
        in_maps.append({
            "up": cu(build_up(u[c * BC:(c + 1) * BC, :T, :], T)),
            "wpack": cb(wpack), "fpack": fpack,
        })
    res = run_bass_kernel_spmd(nc, in_maps, core_ids=list(range(NCORES)),
                               trace=_trace)
    outs = [res.results[c]["out"] for c in range(NCORES)]
    full = np.concatenate([np.asarray(o).T for o in outs], axis=0)
    kernel.last_results = res
    return full.astype(np.float32)


# revision 16
# speedup vs baseline: 1.0015x; 1.0015x over previous
"""Trainium2 Bass kernel for nn_MESNReadout (multi-layer echo state network readout).

Strategy
--------
Pure data parallelism over batch: B=512 -> 64 rows per core on 8 cores; all
weights replicated; output gathered on host.

The reference is a T=1024 sequential scan with L=3 stacked reservoir layers
plus a leaky-integrator side state xv. We reformulate with a *layer-skewed
wavefront*: wavefront k computes x0(k), x1(k-1), x2(k-2), hv(k-3)
simultaneously, where hv(t) = tanh(zv(t)) is the inner tanh of the xv
update. Every input a wavefront needs then comes from the previous
wavefront's tanh output T_{k-1} plus a staged history [x0(k-4); x1(k-4);
x2(k-4)] for the xv pooling term (DVE copies aggregate three rb slots so
PE sees ONE pool matmul; three tiny matmuls instead serialize on the PE
h64 column group at ~200ns each and blow the period).

One wavefront is:
  PE:  projA/projB (input projections, PSUM slot init, prefetched PF ahead)
       mm_b  (pool history -> zv rows, off critical path)
       mm_a  (recurrent matmul, the only op on the dependent chain)
  ACT: one tanh PSUM->SBUF (bf16 out)
  DVE: three small history copies (a wavefront of slack)

The critical cycle is mm_a -> tanh -> mm_a: the minimal PE->ACT->PE round
trip this recurrence permits (~640ns steady: 52 sem + 215 matmul + 58 sem
+ 314 tanh, all but ~95ns of which is fixed engine pipeline latency).
Everything is bf16 so fp32's 2-pass matmuls are avoided and the bigwa
weight reload stays off the chain (the tanh wait rides on the MATMUL,
letting LDWEIGHTS dual-issue under the preceding matmuls). Splitting the
batch columns to pipeline tanh/matmul halves does NOT work: a PSUM bank
read while the bank's accumulation group is open serializes against the
other half's matmul, doubling the round trips per wavefront.

State layout is transposed ([feature, batch]) so matmuls contract over
partitions, and *padded* to partition-aligned blocks x0@[0:20] x1@[32:52]
x2@[64:84] hv@[96:108] because engines can only address SBUF partition
ranges starting at 0/32/64/96 and matmul outputs must start at PSUM
partition 0/32/64/96. Gap rows carry zeros (weights are zero-padded). The
host pre-packs u into a paired time-shifted array up[128, T+5, 64] (rows
0:64 = uT(j-2), rows 64:128 = uT(j-3)) so one projection matmul covers two
skewed time blocks and boundary conditions fall out as zeros.
"""
import sys

import numpy as np

sys.path.insert(0, "/opt/trn_rl_repo")

L, S, TH, D = 3, 4, 5, 64
NCLS = 100
B = 512
DELTA = 0.9
NCORES = 8
BC = B // NCORES            # 64 batch rows per core
R = L * S * TH              # 60
LS = L * S                  # 12
F = R + LS                  # 72 logical state rows
SS = 108                    # padded state span
NB = 6                      # rotating state buffers
NS = 8                      # rotating PSUM slots: one full 2KB bank each
PF = 4                      # projection prefetch distance (slots ahead)
UCHUNK = 16                 # timesteps of `up` per DMA chunk

# padded positions of the 72 logical rows [x0(20) x1(20) x2(20) hv(12)]
NEWPOS = np.concatenate([np.arange(0, 20), np.arange(32, 52),
                         np.arange(64, 84), np.arange(96, 108)])


def _bd(Ws):
    a, b = Ws.shape[1], Ws.shape[2]
    M = np.zeros((S * a, S * b), np.float32)
    for s in range(S):
        M[s * a:(s + 1) * a, s * b:(s + 1) * b] = Ws[s]
    return M


def _hstack_s(Ws):
    return np.concatenate([Ws[s] for s in range(S)], axis=1).astype(np.float32)


def build_host_mats(W_in0, W_in_rest, W, Wv_in, Wv, W_out):
    MpT = np.zeros((LS, R), np.float32)
    for d in range(L):
        for s in range(S):
            MpT[4 * d + s, 20 * d + 5 * s:20 * d + 5 * s + TH] = 1.0 / TH

    # compact [72,72] recurrent matrix in logical order [x0 x1 x2 hv]
    Wc = np.zeros((F, F), np.float32)
    Wc[0:20, 0:20] = _bd(W[0])
    Wc[0:20, 20:40] = _bd(W_in_rest[0][:, D:, :])
    Wc[20:40, 20:40] = _bd(W[1])
    Wc[20:40, 40:60] = _bd(W_in_rest[1][:, D:, :])
    Wc[40:60, 40:60] = _bd(W[2])
    Wc[60:72, 60:72] = DELTA * Wv.T
    BigWa = np.zeros((SS, SS), np.float32)
    BigWa[np.ix_(NEWPOS, NEWPOS)] = Wc

    # input projections: WA -> out rows [0:52] = [U0 | gap | U1],
    # WB -> out rows [64:108] = [U2 | gap | Uv]
    WA = np.zeros((128, 52), np.float32)
    WA[0:64, 0:20] = _hstack_s(W_in0)
    WA[64:128, 32:52] = _hstack_s(W_in_rest[0][:, :D, :])
    WB = np.zeros((128, 44), np.float32)
    WB[0:64, 0:20] = _hstack_s(W_in_rest[1][:, :D, :])
    WB[64:128, 32:44] = Wv_in.T.astype(np.float32)

    # pool-history -> zv: out rows [64:108], cols 32:44 live
    Gw = ((1.0 - DELTA) * (Wv @ MpT)).T.astype(np.float32)   # [60, 12]
    gpool = np.zeros((96, 44), np.float32)
    gpool[0:20, 32:44] = Gw[0:20]
    gpool[32:52, 32:44] = Gw[20:40]
    gpool[64:84, 32:44] = Gw[40:60]

    # xv(T-1) = 0.1*pool(x(T-1)) + 0.9*hv(T-1) over padded feats rows
    poolhv = np.zeros((SS, LS), np.float32)
    poolhv[NEWPOS[0:60], :] = (1.0 - DELTA) * MpT.T
    poolhv[96:108, :] = DELTA * np.eye(LS, dtype=np.float32)

    woutp = np.zeros((SS, NCLS), np.float32)
    woutp[NEWPOS, :] = W_out.astype(np.float32)

    # pack all weights (incl. the bf16 readout mats) into one tensor so
    # startup is 2 DMA issues, not 7
    wpack = np.zeros((128, 360), np.float32)
    wpack[0:SS, 0:108] = BigWa
    wpack[0:96, 108:152] = gpool
    wpack[:, 152:204] = WA
    wpack[:, 204:248] = WB
    wpack[0:SS, 248:260] = poolhv
    wpack[0:SS, 260:360] = woutp
    fpack = np.zeros((128, 1), np.float32)  # b_out, filled by caller
    return wpack, fpack


def build_up(u_core, T):
    """u_core [BC, T, 64] -> up [128, T+5, BC] f32 (paired, shifted, padded)."""
    uT = np.ascontiguousarray(u_core.transpose(2, 1, 0)).astype(np.float32)
    up = np.zeros((128, T + 5, u_core.shape[0]), np.float32)
    up[0:64, 2:T + 2] = uT
    up[64:128, 3:T + 3] = uT
    return np.ascontiguousarray(up)


def build_nc(T, prec="bf16all", split=1):
    import concourse.bacc as bacc
    import concourse.mybir as mybir
    from concourse.tile import TileContext

    dt = mybir.dt.float32
    dtb = mybir.dt.bfloat16 if prec in ("bf16", "bf16all") else mybir.dt.float32
    dtu = mybir.dt.bfloat16 if prec == "bf16all" else mybir.dt.float32
    NW = T + 3
    NUP = T + 5

    nc = bacc.Bacc(None)
    up_d = nc.dram_tensor("up", [128, NUP, BC], dtu, kind="ExternalInput")
    wpack_d = nc.dram_tensor("wpack", [128, 360], dtb, kind="ExternalInput")
    fpack_d = nc.dram_tensor("fpack", [128, 1], dt, kind="ExternalInput")
    out_d = nc.dram_tensor("out", [NCLS, BC], dt, kind="ExternalOutput")

    with TileContext(nc) as tc:
        with (
            tc.tile_pool(name="const", bufs=1) as cpool,
            tc.tile_pool(name="ubuf", bufs=6) as upool,
            tc.tile_pool(name="state", bufs=1) as spool,
            tc.tile_pool(name="psum", bufs=1, space="PSUM") as ppool,
        ):
            wpack = cpool.tile([128, 360], dtb)
            fpack = cpool.tile([128, 1], dt)
            nc.sync.dma_start(wpack[:], wpack_d[:])
            # fpack is tail-only; issue it on ACT (idle at startup) so the
            # first u chunks aren't queued behind it on sync/gpsimd
            nc.scalar.dma_start(fpack[:], fpack_d[:])
            bigwa = wpack[0:SS, 0:108]
            gpool = wpack[0:96, 108:152]
            wa = wpack[:, 152:204]
            wb = wpack[:, 204:248]
            poolhv = wpack[0:SS, 248:260]
            wout = wpack[0:SS, 260:360]
            bout = fpack[0:NCLS, 0:1]

            # one PSUM region: slot j = one full 2KB bank, cols 0:BC used.
            # Only the alignment-gap rows (never written by any matmul but
            # read by the tanh) need zeroing; matmul start=True covers the
            # rest. These memsets gate the first projection (PSUM write
            # ordering), so issue them first on Vector.
            psum = ppool.tile([128, NS, 512], dt)
            nc.vector.memset(psum[32:64, :, 0:BC], 0.0)
            nc.vector.memset(psum[64:96, :, 0:BC], 0.0)

            # rb[:, j%NB, :] = T_{j-1} (tanh output of wavefront j-1), padded
            rb = spool.tile([SS, NB, BC], dtb)
            # hist[:, j%NB, :] = [x0(j-4) | gap | x1(j-4) | gap | x2(j-4)]
            hist = spool.tile([96, NB, BC], dtb)
            nc.vector.memset(rb[:], 0.0)
            nc.vector.memset(hist[:], 0.0)

            # variable-size chunks: small at the head so wavefront 0 isn't
            # gated on a large DMA
            chunks = []
            j = 0
            for w in (2, 2, 4, 8):
                if j < NUP:
                    chunks.append((j, min(w, NUP - j)))
                    j += w
            while j < NUP:
                w = min(UCHUNK, NUP - j)
                chunks.append((j, w))
                j += w
            j2c = {}
            for ci, (j0, w) in enumerate(chunks):
                for jj in range(j0, j0 + w):
                    j2c[jj] = ci
            u_tiles = [None] * len(chunks)
            # keep DMA issuance off ACT (on the critical chain) and PE.
            # The first two chunks gate wavefront 0: put them on gpsimd so
            # they race the wpack DMA on sync instead of queuing behind it.
            dma_eng = [nc.sync, nc.gpsimd]
            next_load = [0]

            def ensure_loaded(jmax):
                while (next_load[0] < len(chunks)
                       and chunks[next_load[0]][0] <= jmax):
                    ci = next_load[0]
                    j0, w = chunks[ci]
                    t = upool.tile([128, UCHUNK, BC], dtu, tag="uc")
                    eng = nc.gpsimd if ci < 2 else dma_eng[ci % 2]
                    eng.dma_start(t[:, :w, :], up_d[:, j0:j0 + w, :])
                    u_tiles[ci] = t
                    next_load[0] += 1

            def up_ap(j):
                ci = j2c[j]
                return u_tiles[ci][:, j - chunks[ci][0], :]

            def emit_proj(k):
                if k >= NW:
                    return
                sl = psum[:, k % NS, 0:BC]
                nc.tensor.matmul(sl[0:52, :], wa, up_ap(k + 2),
                                 start=True, stop=False, skip_group_check=True)
                nc.tensor.matmul(sl[64:108, :], wb, up_ap(k),
                                 start=True, stop=False, skip_group_check=True)

            ensure_loaded(PF + 2 + 2 * UCHUNK)
            for k in range(PF):
                emit_proj(k)

            HB = BC // split
            for k in range(NW):
                ensure_loaded(k + PF + 2 + 2 * UCHUNK)
                emit_proj(k + PF)
                sl = psum[:, k % NS, 0:BC]
                # xv pooling term from staged history (off critical path).
                # One matmul: three tiny matmuls from older rb slots instead
                # serialize on the PE h64 column group and blow the period.
                nc.tensor.matmul(sl[64:108, :], gpool, hist[:, k % NB, :],
                                 start=False, stop=False, skip_group_check=True)
                # the recurrent matmul + tanh, in `split` batch-column
                # halves so the tanh of one half overlaps the matmul of
                # the next (the dependent chain is per batch column)
                for h in range(split):
                    cs = slice(h * HB, (h + 1) * HB)
                    nc.tensor.matmul(sl[0:SS, cs], bigwa,
                                     rb[:, k % NB, cs],
                                     start=False, stop=(h == split - 1),
                                     skip_group_check=True)
                    nc.scalar.activation(rb[:, (k + 1) % NB, cs],
                                         sl[0:SS, cs],
                                         mybir.ActivationFunctionType.Tanh)
                # stage history: x0/x1 two slots ahead (extra slack),
                # x2 one ahead (its source is only ready then)
                if k + 2 < NW:
                    nc.vector.tensor_copy(hist[0:20, (k + 2) % NB, :],
                                          rb[0:20, (k - 1) % NB, :])
                    nc.vector.tensor_copy(hist[32:52, (k + 2) % NB, :],
                                          rb[32:52, k % NB, :])
                if k + 1 < NW:
                    nc.vector.tensor_copy(hist[64:84, (k + 1) % NB, :],
                                          rb[64:84, k % NB, :])

            # ---- tail: feats = [x0|x1|x2|xv](T-1) padded, then readout ----
            feats = spool.tile([SS, BC], dtb)
            nc.vector.memset(feats[:], 0.0)
            nc.vector.tensor_copy(feats[0:20, :], rb[0:20, T % NB, :])
            nc.vector.tensor_copy(feats[32:52, :], rb[32:52, (T + 1) % NB, :])
            nc.vector.tensor_copy(feats[64:84, :], rb[64:84, (T + 2) % NB, :])
            nc.vector.tensor_copy(feats[96:108, :], rb[96:108, (T + 3) % NB, :])
            nc.tensor.matmul(psum[0:LS, 0, 0:BC], poolhv, feats[0:SS, :],
                             start=True, stop=True, skip_group_check=True)
            nc.vector.tensor_copy(feats[96:108, :], psum[0:LS, 0, 0:BC])
            nc.tensor.matmul(psum[0:NCLS, 1, 0:BC], wout, feats[0:SS, :],
                             start=True, stop=True, skip_group_check=True)
            out_sb = spool.tile([NCLS, BC], dt)
            nc.scalar.activation(out_sb[:], psum[0:NCLS, 1, 0:BC],
                                 mybir.ActivationFunctionType.Identity,
                                 bias=bout)
            nc.sync.dma_start(out_d[:], out_sb[:])

    nc.compile()
    return nc


_NC_CACHE = {}


def _get_nc(T, prec="bf16all", split=1):
    key = (T, prec, split)
    if key not in _NC_CACHE:
        _NC_CACHE[key] = build_nc(T, prec, split)
    return _NC_CACHE[key]


def kernel(u, W_in0, W_in_rest, W, Wv_in, Wv, W_out, b_out,
           _T=None, _trace=False, _prec="bf16all", _split=1):
    from concourse.bass_utils import run_bass_kernel_spmd
    import ml_dtypes

    u = np.asarray(u, np.float32)
    T = _T or u.shape[1]
    cb = (lambda x: np.ascontiguousarray(x.astype(ml_dtypes.bfloat16))) \
        if _prec in ("bf16", "bf16all") else (lambda x: x)
    cu = (lambda x: np.ascontiguousarray(x.astype(ml_dtypes.bfloat16))) \
        if _prec == "bf16all" else (lambda x: x)
    wpack, fpack = build_host_mats(
        np.asarray(W_in0, np.float32), np.asarray(W_in_rest, np.float32),
        np.asarray(W, np.float32), np.asarray(Wv_in, np.float32),
        np.asarray(Wv, np.float32), np.asarray(W_out, np.float32))
    fpack = fpack.copy()
    fpack[0:NCLS, 0] = np.asarray(b_out, np.float32)

    nc = _get_nc(T, _prec, _split)
    in_maps = []
    for c in range(NCORES):
        in_maps.append({
            "up": cu(build_up(u[c * BC:(c + 1) * BC, :T, :], T)),
            "wpack": cb(wpack), "fpack": fpack,
        })
    res = run_bass_kernel_spmd(nc, in_maps, core_ids=list(range(NCORES)),
                               trace=_trace)
    outs = [res.results[c]["out"] for c in range(NCORES)]
    full = np.concatenate([np.asarray(o).T for o in outs], axis=0)
    kernel.last_results = res
    return full.astype(np.float32)
